# revision 8
# baseline (speedup 1.0000x reference)
"""Bass/Trainium2 kernel for nn_DegeneratePool: out = x / (H*W + 1e-9).

Pure elementwise scale of a (32, 64, 224, 224) f32 tensor. Data-parallel
across 8 NeuronCores: 4 batches (~51.4 MB) per core, streamed through SBUF
in large contiguous tiles (memory-bound; HWDGE DMAs, DVE multiply).
"""

import numpy as np

import concourse.bacc as bacc
import concourse.mybir as mybir
from concourse.bass_utils import run_bass_kernel_spmd
from concourse.tile import TileContext

N_CORES = 8
B, C, H, W = 32, 64, 224, 224
SCALE = 1.0 / (H * W + 1e-9)

PER_CORE_ELEMS = (B // N_CORES) * C * H * W  # 12,845,056
P = 128
FREE = PER_CORE_ELEMS // P  # 100,352


def _build_nc(
    variant: str = "base",
    tile_f: int = 6272,
    bufs: int = 4,
    repeats: int = 1,
    timing_internal: bool = False,
) -> bacc.Bacc:
    ntiles = FREE // tile_f
    assert ntiles * tile_f == FREE, (tile_f, FREE)
    nc = bacc.Bacc("TRN2", target_bir_lowering=False, num_devices=N_CORES)
    if timing_internal:
        # Timing harness build: tiny external I/O (so the PJRT/axon transfer
        # overhead vanishes) + full-size internal DRAM tensors that the body
        # streams with exactly the production access pattern.
        x_ext = nc.dram_tensor("x", [P, 512], mybir.dt.float32, kind="ExternalInput")
        y_ext = nc.dram_tensor("y", [P, 512], mybir.dt.float32, kind="ExternalOutput")
        x = nc.dram_tensor("xi", [ntiles, P, tile_f], mybir.dt.float32)
        y = nc.dram_tensor("yi", [ntiles, P, tile_f], mybir.dt.float32)
    else:
        x = nc.dram_tensor(
            "x", [ntiles, P, tile_f], mybir.dt.float32, kind="ExternalInput"
        )
        y = nc.dram_tensor(
            "y", [ntiles, P, tile_f], mybir.dt.float32, kind="ExternalOutput"
        )

    with TileContext(nc) as tc:
        if timing_internal:
            with tc.tile_pool(name="io", bufs=1) as io_pool:
                t_io = io_pool.tile([P, 512], mybir.dt.float32)
                nc.sync.dma_start(out=t_io[:], in_=x_ext[:])
                nc.vector.tensor_scalar_mul(t_io[:], t_io[:], SCALE)
                nc.sync.dma_start(out=y_ext[:], in_=t_io[:])
        with tc.tile_pool(name="sbuf", bufs=bufs) as pool:
            for _ in range(repeats):
                for i in range(ntiles):
                    if variant == "base":
                        t = pool.tile([P, tile_f], mybir.dt.float32)
                        nc.sync.dma_start(out=t[:], in_=x[i])
                        nc.vector.tensor_scalar_mul(t[:], t[:], SCALE)
                        nc.sync.dma_start(out=y[i], in_=t[:])
                    elif variant == "scalar_store":
                        # loads on SP HWDGE ring, stores on ACT HWDGE ring
                        t = pool.tile([P, tile_f], mybir.dt.float32)
                        nc.sync.dma_start(out=t[:], in_=x[i])
                        nc.vector.tensor_scalar_mul(t[:], t[:], SCALE)
                        nc.scalar.dma_start(out=y[i], in_=t[:])
                    elif variant == "act_mul":
                        # multiply on the scalar (ACT) engine instead of DVE
                        t = pool.tile([P, tile_f], mybir.dt.float32)
                        nc.sync.dma_start(out=t[:], in_=x[i])
                        nc.scalar.mul(t[:], t[:], SCALE)
                        nc.sync.dma_start(out=y[i], in_=t[:])
                    elif variant == "split_rings":
                        t = pool.tile([P, tile_f], mybir.dt.float32)
                        ld = nc.sync if i % 2 == 0 else nc.scalar
                        st = nc.scalar if i % 2 == 0 else nc.sync
                        ld.dma_start(out=t[:], in_=x[i])
                        nc.vector.tensor_scalar_mul(t[:], t[:], SCALE)
                        st.dma_start(out=y[i], in_=t[:])
                    elif variant == "gpsimd":
                        t = pool.tile([P, tile_f], mybir.dt.float32)
                        nc.gpsimd.dma_start(out=t[:], in_=x[i])
                        nc.vector.tensor_scalar_mul(t[:], t[:], SCALE)
                        nc.gpsimd.dma_start(out=y[i], in_=t[:])
                    else:
                        raise ValueError(variant)
    nc.compile()
    return nc


def kernel(x: np.ndarray) -> np.ndarray:
    assert x.shape == (B, C, H, W) and x.dtype == np.float32
    nc = _build_nc()
    per_core = B // N_CORES
    ntiles = FREE // 6272
    shards = np.ascontiguousarray(x).reshape(N_CORES, ntiles, P, 6272)
    in_maps = [{"x": shards[i]} for i in range(N_CORES)]
    res = run_bass_kernel_spmd(nc, in_maps, core_ids=list(range(N_CORES)))
    out = np.concatenate(
        [r["y"].reshape(per_core, C, H, W) for r in res.results], axis=0
    )
    return out


# revision 10
# speedup vs baseline: 3.5948x; 3.5948x over previous
"""Bass/Trainium2 kernel for nn_DegeneratePool: out = x / (H*W + 1e-9).

The reference collapses to an elementwise scale of a (32, 64, 224, 224) f32
tensor. Data-parallel across 8 NeuronCores: 4 batches (~51.4 MB) per core.
Each core streams its shard through SBUF in 16 contiguous [128, 6272] f32
tiles (3.2 MB per DMA): HWDGE loads on the SP ring, multiply on the vector
engine (DVE, f32 2x mode), HWDGE stores on the ACT ring. Separate rings keep
the store's wait-on-multiply off the sequencer that issues loads, so the
load stream never stalls behind a dependent store (head-of-line blocking).
Memory-bound: ~103 MB of HBM traffic per core at ~358 GB/s -> ~290 us.
"""

import numpy as np

import concourse.bacc as bacc
import concourse.mybir as mybir
from concourse.bass_utils import run_bass_kernel_spmd
from concourse.tile import TileContext

N_CORES = 8
B, C, H, W = 32, 64, 224, 224
SCALE = 1.0 / (H * W + 1e-9)

PER_CORE_ELEMS = (B // N_CORES) * C * H * W  # 12,845,056
P = 128
FREE = PER_CORE_ELEMS // P  # 100,352
TILE_F = 6272
NTILES = FREE // TILE_F  # 16
BUFS = 4


def _build_nc(
    variant: str = "scalar_store",
    tile_f: int = TILE_F,
    bufs: int = BUFS,
    repeats: int = 1,
    timing_internal: bool = False,
) -> bacc.Bacc:
    ntiles = FREE // tile_f
    assert ntiles * tile_f == FREE, (tile_f, FREE)
    nc = bacc.Bacc("TRN2", target_bir_lowering=False, num_devices=N_CORES)
    if timing_internal:
        x_ext = nc.dram_tensor("x", [P, 512], mybir.dt.float32, kind="ExternalInput")
        y_ext = nc.dram_tensor("y", [P, 512], mybir.dt.float32, kind="ExternalOutput")
        x = nc.dram_tensor("xi", [ntiles, P, tile_f], mybir.dt.float32)
        y = nc.dram_tensor("yi", [ntiles, P, tile_f], mybir.dt.float32)
    else:
        x = nc.dram_tensor(
            "x", [ntiles, P, tile_f], mybir.dt.float32, kind="ExternalInput"
        )
        y = nc.dram_tensor(
            "y", [ntiles, P, tile_f], mybir.dt.float32, kind="ExternalOutput"
        )

    with TileContext(nc) as tc:
        if timing_internal:
            with tc.tile_pool(name="io", bufs=1) as io_pool:
                t_io = io_pool.tile([P, 512], mybir.dt.float32)
                nc.sync.dma_start(out=t_io[:], in_=x_ext[:])
                nc.vector.tensor_scalar_mul(t_io[:], t_io[:], SCALE)
                nc.sync.dma_start(out=y_ext[:], in_=t_io[:])
        with tc.tile_pool(name="sbuf", bufs=bufs) as pool:
            for _ in range(repeats):
                for i in range(ntiles):
                    t = pool.tile([P, tile_f], mybir.dt.float32)
                    if variant == "base":
                        nc.sync.dma_start(out=t[:], in_=x[i])
                        nc.vector.tensor_scalar_mul(t[:], t[:], SCALE)
                        nc.sync.dma_start(out=y[i], in_=t[:])
                    elif variant == "scalar_store":
                        nc.sync.dma_start(out=t[:], in_=x[i])
                        nc.vector.tensor_scalar_mul(t[:], t[:], SCALE)
                        nc.scalar.dma_start(out=y[i], in_=t[:])
                    elif variant == "split_rings":
                        ld = nc.sync if i % 2 == 0 else nc.scalar
                        st = nc.scalar if i % 2 == 0 else nc.sync
                        ld.dma_start(out=t[:], in_=x[i])
                        nc.vector.tensor_scalar_mul(t[:], t[:], SCALE)
                        st.dma_start(out=y[i], in_=t[:])
                    elif variant == "act_mul":
                        nc.sync.dma_start(out=t[:], in_=x[i])
                        nc.scalar.mul(t[:], t[:], SCALE)
                        nc.sync.dma_start(out=y[i], in_=t[:])
                    else:
                        raise ValueError(variant)
    nc.compile()
    return nc


_NC_CACHE = {}


def kernel(x: np.ndarray) -> np.ndarray:
    assert tuple(x.shape) == (B, C, H, W)
    x = np.ascontiguousarray(x, dtype=np.float32)
    if "nc" not in _NC_CACHE:
        _NC_CACHE["nc"] = _build_nc()
    nc = _NC_CACHE["nc"]
    per_core = B // N_CORES
    shards = x.reshape(N_CORES, NTILES, P, TILE_F)
    in_maps = [{"x": shards[i]} for i in range(N_CORES)]
    res = run_bass_kernel_spmd(nc, in_maps, core_ids=list(range(N_CORES)))
    out = np.concatenate(
        [r["y"].reshape(per_core, C, H, W) for r in res.results], axis=0
    )
    return out
